# revision 11
# baseline (speedup 1.0000x reference)
"""Biaffine kernel for Trainium2, data-parallel over batch across 8 NeuronCores.

Reference math (per batch b):
    Daug = [D, 1]                                  # [S, d+1]
    out  = Daug @ U @ H^T + (Daug @ W[:d+1])[:, None] + (H @ W[d+1:])[None, :]

Algebraic refactor used here (d = 1024):
    U0 = U[:d]                # [d, d]
    c  = U[d] + W[d+1:]       # [d]  (folds the ones-row of Daug and the H linear term)
    T' = D @ U0 + c           # [S, d]
    dlin = D @ W[:d] + W[d]   # [S]  (tiny; computed host-side)
    out  = T' @ H^T + dlin[:, None]

Device kernel per core (4 batches, 384 matmuls):
    matmul1: T'^T[j, x] = sum_k U0[k, j] * D^T[k, x]  (lhsT = U0, rhs = D^T)
             + per-partition bias c fused into the PSUM->SBUF copy (DVE)
    matmul2: out[x, y] = sum_j T'^T[j, x] * H^T[j, y] (lhsT = T'^T, rhs = H^T)
             + per-partition bias dlin fused into the PSUM->SBUF copy (DVE)

Performance structure:
  - Matmul operands are bfloat16 (fp32 PSUM accumulation): single-pass PE at
    1 cycle/row, same PE rate as float32r but HALF the DMA bytes. The PE
    streaming floor is 384 matmuls x 512 rows @ 2.4 GHz = 81.9 us; input
    DMA demand drops to ~100 GB/s/ring so the PE never starves after start.
    End-to-end relative error ~3.3e-3, entirely from bf16 input/T'
    quantization (validated against a host emulation of the rounding).
  - D^T / H^T / U0 are transposed AND pre-swizzled host-side to the exact SBUF
    layout, so the device does zero transposes and every DMA partition read is
    one contiguous block.
  - Batch 0 is loaded as per-kt chunks, issued in consumption order across
    both HWDGE rings (sync: U0; scalar: D^T), so the first matmul starts as
    soon as u0[0]+dt[0] land (~128 KB each); batch 0's matmul1 runs kt-outer
    across 8 PSUM banks so the PE consumes chunks at DMA arrival rate.
    Batches 1-3 load D^T/H^T as ONE DMA each (fewer descriptors/semaphores).
  - A few bf16 warm-up matmuls on a memset tile occupy the PE during the
    startup DMA so the HAM clock gate is warm when real matmuls begin.
  - Stores alternate rings (xt even: scalar, xt odd: sync) to balance ring
    bytes; the final output tile is computed as two N=256 half-chains so its
    DVE+store overlaps the last matmuls (shorter drain tail).

BIAFFINE_MM=f32r switches to fp32r matmuls (rel err ~2e-4, slower startup);
BIAFFINE_MM=f32 switches to exact fp32 matmuls (~3x slower, rel err ~5e-7).
"""
import os
import sys

import numpy as np

for _p in (
    "/root/.axon_site",
    "/root/.axon_site/_ro/trn_rl_repo",
    "/root/.axon_site/_ro/pypackages",
    "/opt/trn_rl_repo",
):
    if os.path.isdir(_p) and _p not in sys.path:
        sys.path.append(_p)

import concourse.bass as bass
import concourse.mybir as mybir
import concourse.tile as tile
from concourse import bacc
from concourse.bass_utils import run_bass_kernel_spmd

B, S, D_DIM = 32, 512, 1024
N_CORES = 8
BPC = B // N_CORES  # batches per core
KT = D_DIM // 128  # 8 k-tiles (contraction over d)
JT = D_DIM // 128  # 8 j-tiles (M dim of matmul1)
XT = S // 128  # 4 x-tiles (M dim of matmul2)

_NC_CACHE = {}


def _mode() -> str:
    m = os.environ.get("BIAFFINE_MM", "f16")
    assert m in ("f16", "bf16", "f32r", "f32"), m
    return m


def _build_nc(mode: str) -> bass.Bass:
    nc = bacc.Bacc()
    f32 = mybir.dt.float32
    mm_dt = {
        "f16": mybir.dt.float16,
        "bf16": mybir.dt.bfloat16,
        "f32r": mybir.dt.float32r,
        "f32": f32,
    }[mode]
    n_warm = int(os.environ.get("BIAFFINE_WARM", "8"))
    # Store the output in the matmul dtype (fp16/bf16): halves store traffic
    # and the drain-tail store; the host upconverts to fp32. Output range
    # (|out| < ~200) is far inside fp16 range; adds ~2.4e-4 quantization.
    out_dt = mm_dt if mode in ("f16", "bf16") else f32

    # Inputs arrive pre-swizzled to the SBUF layout: [.., p, kt, x] so each
    # partition's DMA read is one contiguous block.
    dt_in = nc.dram_tensor("dt_in", [BPC, 128, KT, S], mm_dt, kind="ExternalInput")
    ht_in = nc.dram_tensor("ht_in", [BPC, 128, KT, S], mm_dt, kind="ExternalInput")
    u0_in = nc.dram_tensor("u0_in", [128, KT, D_DIM], mm_dt, kind="ExternalInput")
    ccol_in = nc.dram_tensor("ccol_in", [128, JT], f32, kind="ExternalInput")
    dcol_in = nc.dram_tensor("dcol_in", [128, BPC * XT], f32, kind="ExternalInput")
    out_t = nc.dram_tensor("out", [BPC, S, S], out_dt, kind="ExternalOutput")

    with tile.TileContext(nc) as tc:
        with (
            tc.tile_pool(name="const", bufs=1) as cpool,
            tc.tile_pool(name="b0", bufs=1) as b0_pool,
            tc.tile_pool(name="dh", bufs=2) as dh_pool,
            tc.tile_pool(name="tt", bufs=2) as tt_pool,
            tc.tile_pool(name="ot", bufs=3) as ot_pool,
            tc.tile_pool(name="ps", bufs=8, space="PSUM") as ps_pool,
        ):
            # HAM warm-up: a few matmuls on a memset tile fill the startup DMA
            # window with real array work so the PE is at the warm clock when
            # the first data matmul issues. (gpsimd memset: it's idle early.)
            warm_sb = cpool.tile([128, S], mm_dt, name="warm_sb")
            nc.gpsimd.memset(warm_sb[:], 0.0)
            warm_ps = ps_pool.tile([128, S], f32, tag="ps", name="warm_ps")
            for _ in range(n_warm):
                nc.tensor.matmul(
                    warm_ps[:], lhsT=warm_sb[:, :128], rhs=warm_sb[:], start=True,
                    stop=True,
                )

            # U0 as 8 per-kt tiles on the sync ring; batch-0 D^T as 8 per-kt
            # chunks on the scalar ring — issued in consumption order so the
            # first matmul starts as soon as u0[0]+dt[0] land.
            u0_t = [
                cpool.tile([128, D_DIM], mm_dt, tag=f"u0k{kt}", name=f"u0k{kt}")
                for kt in range(KT)
            ]
            dt0_t = [
                b0_pool.tile([128, S], mm_dt, tag=f"dt{kt}", name=f"dt{kt}")
                for kt in range(KT)
            ]
            ht0_t = [
                b0_pool.tile([128, S], mm_dt, tag=f"ht{kt}", name=f"ht{kt}")
                for kt in range(KT)
            ]
            # Strict consumption-order interleave, alternating rings by kt
            # parity so both rings carry ~equal bytes per kt step: batch 0's
            # matmul1 (kt-outer) consumes (u0[k], dt0[k]) pairs at ~1.7 us
            # per step, and its matmul2 needs all ht0 right after — dual-ring
            # delivery at ~110 GB/s/ring just keeps ahead of the PE.
            for kt in range(KT):
                a, b_ = (nc.sync, nc.scalar) if kt % 2 == 0 else (nc.scalar, nc.sync)
                if kt == 0:
                    # Halve the very first u0 chunk so the kt=0/jm<4 matmuls
                    # can start ~1 us earlier (subtile deps).
                    hd = D_DIM // 2
                    a.dma_start(u0_t[0][:, :hd], u0_in[:, 0, :hd])
                    b_.dma_start(dt0_t[0][:], dt_in[0, :, 0, :])
                    a.dma_start(u0_t[0][:, hd:], u0_in[:, 0, hd:])
                    continue
                a.dma_start(u0_t[kt][:], u0_in[:, kt, :])
                b_.dma_start(dt0_t[kt][:], dt_in[0, :, kt, :])
            for kt in range(KT):
                (nc.sync if kt % 2 == 0 else nc.scalar).dma_start(
                    ht0_t[kt][:], ht_in[0, :, kt, :]
                )
            # Bias columns are first needed ~15 us in; keep them behind the
            # startup-critical chunks.
            ccol = cpool.tile([128, JT], f32)
            nc.sync.dma_start(ccol[:], ccol_in[:])
            dcol = cpool.tile([128, BPC * XT], f32)
            nc.scalar.dma_start(dcol[:], dcol_in[:])

            dt_full = ht_full = None
            for b in range(BPC):
                # Prefetch batch b+1 as one DMA per tensor (dt: sync ring,
                # ht: scalar ring); emitted before this batch's stores so the
                # loads aren't queued behind store-data-ready waits.
                nxt_dt, nxt_ht = None, None
                if b + 1 < BPC:
                    nxt_dt = dh_pool.tile([128, KT * S], mm_dt, tag="dtf", name="dtf")
                    nxt_ht = dh_pool.tile([128, KT * S], mm_dt, tag="htf", name="htf")
                    dsrc = dt_in[b + 1].rearrange("p k x -> p (k x)")
                    hsrc = ht_in[b + 1].rearrange("p k x -> p (k x)")
                    hw = KT * S // 2
                    if b == 0:
                        # Batch 1 is consumed right on the heels of the
                        # startup loads: split it so the first half (kt 0-3)
                        # lands before batch-1 matmul1 reaches it (subtile
                        # deps let those matmuls start on the half).
                        nc.sync.dma_start(nxt_dt[:, :hw], dsrc[:, :hw])
                        nc.sync.dma_start(nxt_dt[:, hw:], dsrc[:, hw:])
                        nc.scalar.dma_start(nxt_ht[:, :hw], hsrc[:, :hw])
                        nc.scalar.dma_start(nxt_ht[:, hw:], hsrc[:, hw:])
                    else:
                        nc.sync.dma_start(nxt_dt[:], dsrc)
                        nc.scalar.dma_start(nxt_ht[:], hsrc)

                if b == 0:
                    dt_rhs = [dt0_t[kt][:] for kt in range(KT)]
                    ht_rhs = [ht0_t[kt][:] for kt in range(KT)]
                else:
                    dt_rhs = [dt_full[:, kt * S : (kt + 1) * S] for kt in range(KT)]
                    ht_rhs = [ht_full[:, kt * S : (kt + 1) * S] for kt in range(KT)]

                # matmul1: T'^T[jm*128+p, x]  (+ bias c)
                tt_t = [
                    tt_pool.tile([128, S], mm_dt, tag=f"tt{jm}", name=f"tt{jm}")
                    for jm in range(JT)
                ]
                if b == 0:
                    # kt-outer: 8 live PSUM banks; each kt step needs only
                    # chunk kt of u0/dt, so the PE tracks DMA arrivals.
                    ps_l = [
                        ps_pool.tile([128, S], f32, tag="ps", name=f"ps{jm}")
                        for jm in range(JT)
                    ]
                    for kt in range(KT):
                        for jm in range(JT):
                            nc.tensor.matmul(
                                ps_l[jm][:],
                                lhsT=u0_t[kt][:, jm * 128 : (jm + 1) * 128],
                                rhs=dt_rhs[kt],
                                start=(kt == 0),
                                stop=(kt == KT - 1),
                            )
                    for jm in range(JT):
                        # Alternate DVE/ACT: all 8 banks stop within
                        # ~1.7 us at the end of the kt loop, and one engine
                        # at ~480 ns/copy can't feed matmul2's 216 ns/step
                        # consumption. (gpsimd can't read PSUM; the scalar
                        # engine's DMA-issue work is long done by then.)
                        if jm % 2 == 0:
                            nc.vector.tensor_scalar_add(
                                tt_t[jm][:], ps_l[jm][:], ccol[:, jm : jm + 1]
                            )
                        else:
                            nc.scalar.activation(
                                tt_t[jm][:],
                                ps_l[jm][:],
                                mybir.ActivationFunctionType.Identity,
                                bias=ccol[:, jm : jm + 1],
                            )
                else:
                    for jm in range(JT):
                        ps = ps_pool.tile([128, S], f32, tag="ps", name="ps")
                        for kt in range(KT):
                            nc.tensor.matmul(
                                ps[:],
                                lhsT=u0_t[kt][:, jm * 128 : (jm + 1) * 128],
                                rhs=dt_rhs[kt],
                                start=(kt == 0),
                                stop=(kt == KT - 1),
                            )
                        nc.vector.tensor_scalar_add(
                            tt_t[jm][:], ps[:], ccol[:, jm : jm + 1]
                        )

                # matmul2: out[xt*128+p, y]  (+ bias dlin)
                for xt in range(XT):
                    last_tile = b == BPC - 1 and xt == XT - 1
                    if not last_tile:
                        po = ps_pool.tile([128, S], f32, tag="ps", name="po")
                        for jm in range(JT):
                            nc.tensor.matmul(
                                po[:],
                                lhsT=tt_t[jm][:, xt * 128 : (xt + 1) * 128],
                                rhs=ht_rhs[jm],
                                start=(jm == 0),
                                stop=(jm == JT - 1),
                            )
                        ot = ot_pool.tile([128, S], out_dt, tag="ot", name="ot")
                        nc.vector.tensor_scalar_add(
                            ot[:], po[:], dcol[:, b * XT + xt : b * XT + xt + 1]
                        )
                        # Alternate store rings to balance bytes.
                        eng = nc.scalar if xt % 2 == 0 else nc.sync
                        eng.dma_start(out_t[b, xt * 128 : (xt + 1) * 128, :], ot[:])
                    else:
                        # Final tile: two N=256 half-chains in separate
                        # PSUM banks (a shared bank serializes half B behind
                        # half A's DVE read) so the first half's DVE+store
                        # overlaps the second half's matmuls.
                        for half, eng in ((0, nc.scalar), (1, nc.sync)):
                            lo, hi = half * 256, half * 256 + 256
                            po = ps_pool.tile([128, S], f32, tag="ps", name="po")
                            for jm in range(JT):
                                nc.tensor.matmul(
                                    po[:, lo:hi],
                                    lhsT=tt_t[jm][:, xt * 128 : (xt + 1) * 128],
                                    rhs=ht_rhs[jm][:, lo:hi],
                                    start=(jm == 0),
                                    stop=(jm == JT - 1),
                                )
                            ot = ot_pool.tile([128, 256], out_dt, tag="oth", name="oth")
                            nc.vector.tensor_scalar_add(
                                ot[:], po[:, lo:hi], dcol[:, b * XT + xt : b * XT + xt + 1]
                            )
                            eng.dma_start(
                                out_t[b, xt * 128 : (xt + 1) * 128, lo:hi], ot[:]
                            )

                if nxt_dt is not None:
                    dt_full, ht_full = nxt_dt, nxt_ht
    nc.finalize()
    return nc


def _get_nc() -> bass.Bass:
    key = f"nc_{_mode()}"
    if key not in _NC_CACHE:
        _NC_CACHE[key] = _build_nc(_mode())
    return _NC_CACHE[key]


def _round_fp32r(a: np.ndarray) -> np.ndarray:
    """Round fp32 to fp32r layout: RNE to 11-bit mantissa, low 12 bits zero."""
    bits = np.ascontiguousarray(a, dtype=np.float32).view(np.uint32)
    odd = (bits >> 12) & np.uint32(1)
    out = (bits + np.uint32(0x7FF) + odd) & np.uint32(0xFFFFF000)
    return out.view(np.float32)


def kernel(D, H, U, W):
    D = np.ascontiguousarray(np.asarray(D, dtype=np.float32))
    H = np.ascontiguousarray(np.asarray(H, dtype=np.float32))
    U = np.asarray(U, dtype=np.float32)
    W = np.asarray(W, dtype=np.float32)
    d = D_DIM
    mode = _mode()
    np_mm = np.dtype(
        mybir.dt.np(
            {
                "f16": mybir.dt.float16,
                "bf16": mybir.dt.bfloat16,
                "f32r": mybir.dt.float32r,
                "f32": mybir.dt.float32,
            }[mode]
        )
    )

    def to_mm(a: np.ndarray) -> np.ndarray:
        if mode == "f32r":
            return _round_fp32r(a)
        return np.ascontiguousarray(a).astype(np_mm)

    # U0 swizzled to [128, KT, d]: [p, kt, j] = U0[kt*128+p, j]
    U0 = to_mm(np.ascontiguousarray(U[:d, :].reshape(KT, 128, d).transpose(1, 0, 2)))
    c = (U[d, :] + W[d + 1 :]).astype(np.float32)  # [d]
    # ccol[p, jm] = c[jm*128 + p]
    ccol = np.ascontiguousarray(c.reshape(JT, 128).T)
    # dlin[b, x] = D[b, x] . W[:d] + W[d]  (from unrounded fp32 D: exact)
    dlin = (D @ W[:d] + W[d]).astype(np.float32)  # [B, S]

    in_maps = []
    for cidx in range(N_CORES):
        sl = slice(cidx * BPC, (cidx + 1) * BPC)
        # [b, p, kt, x] = X[b, x, kt*128+p]  (transpose + swizzle in one copy)
        Dt = to_mm(D[sl].reshape(BPC, S, KT, 128).transpose(0, 3, 2, 1))
        Ht = to_mm(H[sl].reshape(BPC, S, KT, 128).transpose(0, 3, 2, 1))
        # dcol[p, b*XT + xt] = dlin[b, xt*128 + p]
        dcol = np.ascontiguousarray(
            dlin[sl].reshape(BPC, XT, 128).transpose(2, 0, 1).reshape(128, BPC * XT)
        )
        in_maps.append(
            {
                "dt_in": Dt,
                "ht_in": Ht,
                "u0_in": U0,
                "ccol_in": ccol,
                "dcol_in": dcol,
            }
        )

    nc = _get_nc()
    trace = bool(int(os.environ.get("BIAFFINE_TRACE", "0")))
    kwargs = {}
    if trace:
        tdir = os.environ.get("BIAFFINE_TRACE_DIR")
        if tdir:
            os.makedirs(tdir, exist_ok=True)
            kwargs["tmpdir"] = tdir
    res = run_bass_kernel_spmd(
        nc, in_maps, core_ids=list(range(N_CORES)), trace=trace, **kwargs
    )
    if trace and res.exec_time_ns is not None:
        print(f"HW exec time: {res.exec_time_ns} ns")

    out = np.concatenate([res.results[i]["out"] for i in range(N_CORES)], axis=0)
    return np.ascontiguousarray(out.astype(np.float32))
